# revision 2
# baseline (speedup 1.0000x reference)
"""Self-contained Trainium2 Bass kernel for a 2-layer GCN (DepGCN).

reference semantics:
    x1 = relu(Ahat @ (x @ W1) + b1)         Ahat = D^-1/2 (A+I) D^-1/2
    x2 = relu(Ahat @ (x1 @ W2) + b2)
    p  = max(x2, axis=0, keepdims=True)
    return (x2, p)

Strategy (8 NeuronCores, SPMD):
  - Shard dst nodes across cores (2500 rows each); edges partitioned by dst.
  - Aggregate-first form: agg = Ahat @ X, then out = relu(agg @ W + b).
    (associativity: Ahat (X W) == (Ahat X) W)
  - The sparse aggregation is done as one-hot matmuls on the PE:
    for each 128-dst block, gather the source rows for its edges with
    dma_gather (rows land on partitions = edge slots), build the selection
    matrix M[e, d] = norm_e * (dst_e == d) on the vector engine
    (iota == dstoff) * norm, and accumulate psum += M.T @ G.
  - Layer 1 gathers from the full x input (replicated in each core's HBM);
    layer 2 gathers from x1_full obtained via an AllGather of the 8 local
    x1 shards.
  - p is computed per-core via running max + PE transpose + free-axis
    reduce_max; host combines the 8 partial maxes.
"""

import numpy as np

N_NODES = 20000
D = 512
N_CORES = 8
NLOC = N_NODES // N_CORES  # 2500
P = 128
NBLK = (NLOC + P - 1) // P  # 20
LAST_ROWS = NLOC - (NBLK - 1) * P  # 68

_NC_CACHE = {}


def _preprocess(edge_index):
    """Build per-core gather indices / selection metadata from the edge list."""
    src = np.asarray(edge_index[0]).astype(np.int64)
    dst = np.asarray(edge_index[1]).astype(np.int64)
    loops = np.arange(N_NODES, dtype=np.int64)
    src_f = np.concatenate([src, loops])
    dst_f = np.concatenate([dst, loops])

    deg = np.bincount(dst_f, minlength=N_NODES).astype(np.float64)
    s = (1.0 / np.sqrt(deg)).astype(np.float32)
    norm = (s[src_f] * s[dst_f]).astype(np.float32)

    order = np.argsort(dst_f, kind="stable")
    ds = dst_f[order]
    ss = src_f[order]
    ns = norm[order]

    # segment boundaries for each (core, block)
    starts = []
    for c in range(N_CORES):
        for b in range(NBLK):
            starts.append(c * NLOC + b * P)
    starts.append(N_NODES)
    bounds = np.searchsorted(ds, np.array(starts, dtype=np.int64))

    counts = bounds[1:] - bounds[:-1]  # [160]
    ncb = int(max(1, int(np.ceil(counts.max() / P))))

    W = ncb * P
    gidx = np.zeros((N_CORES, NBLK, W), dtype=np.int16)
    doff = np.full((N_CORES, NBLK, W), -1.0, dtype=np.float32)
    nrm = np.zeros((N_CORES, NBLK, W), dtype=np.float32)

    for c in range(N_CORES):
        for b in range(NBLK):
            i = c * NBLK + b
            lo, hi = bounds[i], bounds[i + 1]
            k = hi - lo
            if k == 0:
                continue
            gidx[c, b, :k] = ss[lo:hi].astype(np.int16)
            doff[c, b, :k] = (ds[lo:hi] - (c * NLOC + b * P)).astype(np.float32)
            nrm[c, b, :k] = ns[lo:hi]

    # dma_gather index layout: idx i -> partition i%16 (replicated x8), col i//16
    A = gidx.reshape(N_CORES, NBLK, ncb * 8, 16)
    B = A.transpose(0, 1, 3, 2)  # [C, NBLK, 16, ncb*8]
    Bt = np.tile(B, (1, 1, 8, 1))  # [C, NBLK, 128, ncb*8]
    idx_dev = Bt.transpose(0, 2, 1, 3).reshape(N_CORES, P, NBLK * ncb * 8)
    idx_dev = np.ascontiguousarray(idx_dev)

    # meta layout: for chunk (b, cc): col 2*(b*ncb+cc) = dstoff, +1 = norm
    D0 = doff.reshape(N_CORES, NBLK, ncb, P)
    N0 = nrm.reshape(N_CORES, NBLK, ncb, P)
    M4 = np.stack([D0, N0], axis=-1)  # [C, NBLK, ncb, P, 2]
    meta_dev = M4.transpose(0, 3, 1, 2, 4).reshape(N_CORES, P, NBLK * ncb * 2)
    meta_dev = np.ascontiguousarray(meta_dev)

    return ncb, idx_dev, meta_dev


def _build_nc(ncb):
    import concourse.bacc as bacc
    import concourse.mybir as mybir
    import concourse.tile as tile
    from concourse.masks import make_identity

    f32 = mybir.dt.float32
    i16 = mybir.dt.int16

    nc = bacc.Bacc(
        "TRN2",
        target_bir_lowering=False,
        debug=False,
        enable_asserts=False,
        num_devices=N_CORES,
    )

    x_ap = nc.dram_tensor("x", [N_NODES, D], f32, kind="ExternalInput").ap()
    w1_ap = nc.dram_tensor("W1", [D, D], f32, kind="ExternalInput").ap()
    w2_ap = nc.dram_tensor("W2", [D, D], f32, kind="ExternalInput").ap()
    b1_ap = nc.dram_tensor("b1", [D], f32, kind="ExternalInput").ap()
    b2_ap = nc.dram_tensor("b2", [D], f32, kind="ExternalInput").ap()
    idx_ap = nc.dram_tensor("gidx", [P, NBLK * ncb * 8], i16, kind="ExternalInput").ap()
    meta_ap = nc.dram_tensor("meta", [P, NBLK * ncb * 2], f32, kind="ExternalInput").ap()
    x2_ap = nc.dram_tensor("x2", [NLOC, D], f32, kind="ExternalOutput").ap()
    pm_ap = nc.dram_tensor("pmax", [P, 4], f32, kind="ExternalOutput").ap()

    with tile.TileContext(nc) as tc:
        with (
            tc.tile_pool(name="const", bufs=1) as const,
            tc.tile_pool(name="g", bufs=2) as gpool,
            tc.tile_pool(name="m", bufs=4) as mpool,
            tc.tile_pool(name="s", bufs=3) as spool,
            tc.tile_pool(name="t", bufs=2) as tpool,
            tc.tile_pool(name="pag", bufs=2, space="PSUM") as pag,
            tc.tile_pool(name="pden", bufs=2, space="PSUM") as pden,
            tc.tile_pool(name="ptr", bufs=2, space="PSUM") as ptr,
            tc.tile_pool(name="dram", bufs=1, space="DRAM") as dram,
        ):
            ident = const.tile([P, P], f32)
            make_identity(nc, ident[:])
            iota = const.tile([P, P], f32)
            nc.gpsimd.iota(
                iota[:],
                pattern=[[1, P]],
                base=0,
                channel_multiplier=0,
                allow_small_or_imprecise_dtypes=True,
            )
            ones1 = const.tile([1, P], f32)
            nc.vector.memset(ones1[:], 1.0)

            w1sb = const.tile([P, 4, D], f32)
            nc.sync.dma_start(out=w1sb[:], in_=w1_ap.rearrange("(j p) f -> p j f", p=P))
            w2sb = const.tile([P, 4, D], f32)
            nc.sync.dma_start(out=w2sb[:], in_=w2_ap.rearrange("(j p) f -> p j f", p=P))
            b1sb = const.tile([1, D], f32)
            nc.sync.dma_start(out=b1sb[:], in_=b1_ap[None, :])
            b2sb = const.tile([1, D], f32)
            nc.sync.dma_start(out=b2sb[:], in_=b2_ap[None, :])

            idx_sb = const.tile([P, NBLK * ncb * 8], i16)
            nc.sync.dma_start(out=idx_sb[:], in_=idx_ap[:])
            meta_sb = const.tile([P, NBLK * ncb * 2], f32)
            nc.sync.dma_start(out=meta_sb[:], in_=meta_ap[:])

            mrun = const.tile([P, D], f32)
            nc.vector.memset(mrun[:], 0.0)  # post-relu values are >= 0

            x1loc = dram.tile([NLOC, D], f32)
            x1full = dram.tile([N_NODES, D], f32)

            def layer(src_ap, wsb, bsb, is_last):
                for b in range(NBLK):
                    rows = P if b < NBLK - 1 else LAST_ROWS
                    G = gpool.tile([P, ncb, D], f32)
                    nc.gpsimd.dma_gather(
                        G[:],
                        src_ap,
                        idx_sb[:, b * ncb * 8 : (b + 1) * ncb * 8],
                        ncb * P,
                        ncb * P,
                        D,
                        elem_step=D,
                        single_packet=False,
                    )
                    ps_agg = pag.tile([P, D], f32)
                    for cc in range(ncb):
                        mt = mpool.tile([P, P], f32)
                        col = (b * ncb + cc) * 2
                        nc.vector.tensor_scalar(
                            out=mt[:],
                            in0=iota[:],
                            scalar1=meta_sb[:, col : col + 1],
                            scalar2=meta_sb[:, col + 1 : col + 2],
                            op0=mybir.AluOpType.is_equal,
                            op1=mybir.AluOpType.mult,
                        )
                        nc.tensor.matmul(
                            ps_agg[:],
                            lhsT=mt[:],
                            rhs=G[:, cc, :],
                            start=(cc == 0),
                            stop=(cc == ncb - 1),
                        )
                    agg = spool.tile([P, D], f32)
                    nc.scalar.copy(out=agg[:], in_=ps_agg[:])
                    aggT = tpool.tile([P, 4, P], f32)
                    for j in range(4):
                        ps_t = ptr.tile([P, P], f32)
                        nc.tensor.transpose(
                            out=ps_t[:], in_=agg[:, j * P : (j + 1) * P], identity=ident[:]
                        )
                        nc.vector.tensor_copy(out=aggT[:, j, :], in_=ps_t[:])
                    ps_d = pden.tile([P, D], f32)
                    nc.tensor.matmul(
                        ps_d[:], lhsT=ones1[:], rhs=bsb[:], start=True, stop=False
                    )
                    for j in range(4):
                        nc.tensor.matmul(
                            ps_d[:],
                            lhsT=aggT[:, j, :],
                            rhs=wsb[:, j, :],
                            start=False,
                            stop=(j == 3),
                        )
                    o = spool.tile([P, D], f32)
                    nc.scalar.activation(
                        out=o[:], in_=ps_d[:], func=mybir.ActivationFunctionType.Relu
                    )
                    if not is_last:
                        nc.sync.dma_start(
                            out=x1loc[b * P : b * P + rows, :], in_=o[:rows, :]
                        )
                    else:
                        nc.sync.dma_start(
                            out=x2_ap[b * P : b * P + rows, :], in_=o[:rows, :]
                        )
                        nc.vector.tensor_tensor(
                            out=mrun[:rows, :],
                            in0=mrun[:rows, :],
                            in1=o[:rows, :],
                            op=mybir.AluOpType.max,
                        )

            layer(x_ap, w1sb, b1sb, is_last=False)

            nc.gpsimd.collective_compute(
                "AllGather",
                mybir.AluOpType.bypass,
                replica_groups=[list(range(N_CORES))],
                ins=[x1loc[:].opt()],
                outs=[x1full[:].opt()],
            )

            layer(x1full[:], w2sb, b2sb, is_last=True)

            pm_sb = const.tile([P, 4], f32)
            for j in range(4):
                ps_t = ptr.tile([P, P], f32)
                nc.tensor.transpose(
                    out=ps_t[:], in_=mrun[:, j * P : (j + 1) * P], identity=ident[:]
                )
                nc.vector.reduce_max(
                    out=pm_sb[:, j : j + 1], in_=ps_t[:], axis=mybir.AxisListType.X
                )
            nc.sync.dma_start(out=pm_ap[:], in_=pm_sb[:])

    nc.compile()
    return nc


LAST_EXEC_NS = None
LAST_RESULTS = None


def kernel(x, edge_index, W1, b1, W2, b2, _trace=False, _trace_cores=None):
    global LAST_EXEC_NS, LAST_RESULTS
    from concourse.bass_utils import run_bass_kernel_spmd

    x = np.ascontiguousarray(np.asarray(x, dtype=np.float32))
    W1 = np.ascontiguousarray(np.asarray(W1, dtype=np.float32))
    W2 = np.ascontiguousarray(np.asarray(W2, dtype=np.float32))
    b1 = np.ascontiguousarray(np.asarray(b1, dtype=np.float32))
    b2 = np.ascontiguousarray(np.asarray(b2, dtype=np.float32))

    ncb, idx_dev, meta_dev = _preprocess(edge_index)

    if ncb not in _NC_CACHE:
        _NC_CACHE[ncb] = _build_nc(ncb)
    nc = _NC_CACHE[ncb]

    in_maps = []
    for c in range(N_CORES):
        in_maps.append(
            {
                "x": x,
                "W1": W1,
                "W2": W2,
                "b1": b1,
                "b2": b2,
                "gidx": idx_dev[c],
                "meta": meta_dev[c],
            }
        )

    kwargs = {}
    if _trace:
        kwargs["trace"] = True
        if _trace_cores is not None:
            kwargs["trace_cores"] = _trace_cores
    res = run_bass_kernel_spmd(nc, in_maps, core_ids=list(range(N_CORES)), **kwargs)
    LAST_EXEC_NS = res.exec_time_ns
    LAST_RESULTS = res

    x2 = np.concatenate([res.results[c]["x2"] for c in range(N_CORES)], axis=0)
    pm = np.max(np.stack([res.results[c]["pmax"] for c in range(N_CORES)]), axis=0)
    p = pm.T.reshape(1, D).copy()
    return x2, p


# revision 7
# speedup vs baseline: 1.0283x; 1.0283x over previous
"""Self-contained Trainium2 Bass kernel for a 2-layer GCN (DepGCN).

reference semantics:
    x1 = relu(Ahat @ (x @ W1) + b1)         Ahat = D^-1/2 (A+I) D^-1/2
    x2 = relu(Ahat @ (x1 @ W2) + b2)
    p  = max(x2, axis=0, keepdims=True)
    return (x2, p)

Strategy (8 NeuronCores, SPMD):
  - Shard dst nodes across cores (2500 rows each); edges partitioned by dst
    and sorted by dst on the host; only index metadata is computed host-side.
  - Aggregate-first form: agg = Ahat @ X, then out = relu(agg @ W + b)
    (associativity: Ahat (X W) == (Ahat X) W).
  - Normalization is split: Ahat = diag(s) (A+I) diag(s) with s = deg^-1/2.
    The gather source is pre-scaled (s * x, cast to bf16); the dst-side s is
    fused into the PSUM->SBUF copy. The per-edge selection matrix M is then
    EXACT 0/1 one-hot, safe in bf16.
  - Sparse aggregation as one-hot matmuls on the PE: for each 128-dst block,
    dma_gather the bf16 source rows for its (sorted, padded) edges so that
    edge slot e lands on partition e%128, chunk e//128; build
    M[e, d] = (dstoff[e] == d) on the vector engine; accumulate
    psum += M.T @ G over the block's chunks.
  - Dense part: PE-transpose the (scaled) aggregate, 4 k-chunk matmuls
    against W (kept natural layout in SBUF), bias via a K=1 ones-matmul,
    ReLU on the scalar engine (with the next layer's s-scale fused).
  - Layer boundaries: AllGather of the 2500-row bf16 shard gives every core
    the full 20000-row gather source for the next layer.
  - p: running elementwise max + PE transpose + free-axis reduce_max;
    the host combines the 8 per-core partial maxima.
"""

import numpy as np

N_NODES = 20000
D = 512
N_CORES = 8
NLOC = N_NODES // N_CORES  # 2500
P = 128
NBLK = (NLOC + P - 1) // P  # 20
LAST_ROWS = NLOC - (NBLK - 1) * P  # 68

_NC_CACHE = {}


def _preprocess(edge_index):
    """Build per-core gather indices / selection metadata from the edge list."""
    src = np.asarray(edge_index[0]).astype(np.int64)
    dst = np.asarray(edge_index[1]).astype(np.int64)
    loops = np.arange(N_NODES, dtype=np.int64)
    src_f = np.concatenate([src, loops])
    dst_f = np.concatenate([dst, loops])

    deg = np.bincount(dst_f, minlength=N_NODES).astype(np.float64)
    s = (1.0 / np.sqrt(deg)).astype(np.float32)

    order = np.argsort(dst_f, kind="stable")
    ds = dst_f[order]
    ss = src_f[order]

    starts = []
    for c in range(N_CORES):
        for b in range(NBLK):
            starts.append(c * NLOC + b * P)
    starts.append(N_NODES)
    bounds = np.searchsorted(ds, np.array(starts, dtype=np.int64))

    counts = bounds[1:] - bounds[:-1]  # [160]
    ncb = int(max(1, int(np.ceil(counts.max() / P))))

    W = ncb * P
    gidx = np.zeros((N_CORES, NBLK, W), dtype=np.int16)
    doff = np.full((N_CORES, NBLK, W), -1.0, dtype=np.float32)

    for c in range(N_CORES):
        for b in range(NBLK):
            i = c * NBLK + b
            lo, hi = bounds[i], bounds[i + 1]
            k = hi - lo
            if k == 0:
                continue
            gidx[c, b, :k] = ss[lo:hi].astype(np.int16)
            doff[c, b, :k] = (ds[lo:hi] - (c * NLOC + b * P)).astype(np.float32)

    # dma_gather index layout: idx i -> partition i%16 (replicated x8), col i//16
    A = gidx.reshape(N_CORES, NBLK, ncb * 8, 16)
    B = A.transpose(0, 1, 3, 2)  # [C, NBLK, 16, ncb*8]
    Bt = np.tile(B, (1, 1, 8, 1))  # [C, NBLK, 128, ncb*8]
    idx_dev = np.ascontiguousarray(
        Bt.transpose(0, 2, 1, 3).reshape(N_CORES, P, NBLK * ncb * 8)
    )

    # dstoff layout: col (b*ncb + cc) = chunk's per-partition dst offsets
    doff_dev = np.ascontiguousarray(
        doff.reshape(N_CORES, NBLK, ncb, P).transpose(0, 3, 1, 2)
        .reshape(N_CORES, P, NBLK * ncb)
    )

    # per-core per-block s values: sloc[c][p, b] = s[c*NLOC + b*P + p]
    s_pad = np.zeros(N_CORES * NBLK * P, dtype=np.float32)
    for c in range(N_CORES):
        for b in range(NBLK):
            g0 = c * NLOC + b * P
            rows = min(P, NLOC - b * P)
            s_pad[(c * NBLK + b) * P : (c * NBLK + b) * P + rows] = s[g0 : g0 + rows]
    sloc_dev = np.ascontiguousarray(
        s_pad.reshape(N_CORES, NBLK, P).transpose(0, 2, 1)
    )

    return ncb, idx_dev, doff_dev, sloc_dev


def _build_nc(ncb, rep=1, gdt_name="bf16"):
    import concourse.bacc as bacc
    import concourse.mybir as mybir
    import concourse.tile as tile
    from concourse.masks import make_identity

    f32 = mybir.dt.float32
    i16 = mybir.dt.int16
    gdt = mybir.dt.bfloat16 if gdt_name == "bf16" else mybir.dt.float32

    nc = bacc.Bacc(
        "TRN2",
        target_bir_lowering=False,
        debug=False,
        enable_asserts=False,
        num_devices=N_CORES,
    )

    x_ap = nc.dram_tensor("x", [NLOC, D], f32, kind="ExternalInput").ap()
    w1_ap = nc.dram_tensor("W1", [D, D], f32, kind="ExternalInput").ap()
    w2_ap = nc.dram_tensor("W2", [D, D], f32, kind="ExternalInput").ap()
    b1_ap = nc.dram_tensor("b1", [D], f32, kind="ExternalInput").ap()
    b2_ap = nc.dram_tensor("b2", [D], f32, kind="ExternalInput").ap()
    idx_ap = nc.dram_tensor("gidx", [P, NBLK * ncb * 8], i16, kind="ExternalInput").ap()
    doff_ap = nc.dram_tensor("doff", [P, NBLK * ncb], f32, kind="ExternalInput").ap()
    sloc_ap = nc.dram_tensor("sloc", [P, NBLK], f32, kind="ExternalInput").ap()
    x2_ap = nc.dram_tensor("x2", [NLOC, D], f32, kind="ExternalOutput").ap()
    pm_ap = nc.dram_tensor("pmax", [P, 4], f32, kind="ExternalOutput").ap()

    RG = [list(range(N_CORES))]

    with tile.TileContext(nc) as tc:
        with (
            tc.tile_pool(name="const", bufs=1) as const,
            tc.tile_pool(name="g", bufs=3) as gpool,
            tc.tile_pool(name="m", bufs=4) as mpool,
            tc.tile_pool(name="s", bufs=3) as spool,
            tc.tile_pool(name="t", bufs=2) as tpool,
            tc.tile_pool(name="pag", bufs=2, space="PSUM") as pag,
            tc.tile_pool(name="pden", bufs=2, space="PSUM") as pden,
            tc.tile_pool(name="ptr", bufs=2, space="PSUM") as ptr,
            tc.tile_pool(name="dram", bufs=1, space="DRAM") as dram,
        ):
            ident = const.tile([P, P], f32)
            make_identity(nc, ident[:])
            iota = const.tile([P, P], f32)
            nc.gpsimd.iota(
                iota[:],
                pattern=[[1, P]],
                base=0,
                channel_multiplier=0,
                allow_small_or_imprecise_dtypes=True,
            )
            ones1 = const.tile([1, P], f32)
            nc.vector.memset(ones1[:], 1.0)

            w1sb = const.tile([P, 4, D], f32)
            nc.sync.dma_start(out=w1sb[:], in_=w1_ap.rearrange("(j p) f -> p j f", p=P))
            w2sb = const.tile([P, 4, D], f32)
            nc.sync.dma_start(out=w2sb[:], in_=w2_ap.rearrange("(j p) f -> p j f", p=P))
            b1sb = const.tile([1, D], f32)
            nc.sync.dma_start(out=b1sb[:], in_=b1_ap[None, :])
            b2sb = const.tile([1, D], f32)
            nc.sync.dma_start(out=b2sb[:], in_=b2_ap[None, :])

            idx_sb = const.tile([P, NBLK * ncb * 8], i16)
            nc.sync.dma_start(out=idx_sb[:], in_=idx_ap[:])
            doff_sb = const.tile([P, NBLK * ncb], f32)
            nc.sync.dma_start(out=doff_sb[:], in_=doff_ap[:])
            sloc_sb = const.tile([P, NBLK], f32)
            nc.sync.dma_start(out=sloc_sb[:], in_=sloc_ap[:])

            mrun = const.tile([P, D], f32)

            xp_loc = dram.tile([NLOC, D], gdt)
            xp_full = dram.tile([N_NODES, D], gdt)
            x1p_loc = dram.tile([NLOC, D], gdt)
            x1p_full = dram.tile([N_NODES, D], gdt)

            def layer(src_ap, wsb, bsb, is_last, out_loc):
                for b in range(NBLK):
                    rows = P if b < NBLK - 1 else LAST_ROWS
                    G = gpool.tile([P, ncb, D], gdt)
                    nc.gpsimd.dma_gather(
                        G[:],
                        src_ap,
                        idx_sb[:, b * ncb * 8 : (b + 1) * ncb * 8],
                        ncb * P,
                        ncb * P,
                        D,
                        elem_step=D,
                        single_packet=False,
                    )
                    ps_agg = pag.tile([P, D], f32)
                    for cc in range(ncb):
                        mt = mpool.tile([P, P], gdt)
                        col = b * ncb + cc
                        nc.vector.tensor_scalar(
                            out=mt[:],
                            in0=iota[:],
                            scalar1=doff_sb[:, col : col + 1],
                            scalar2=None,
                            op0=mybir.AluOpType.is_equal,
                        )
                        nc.tensor.matmul(
                            ps_agg[:],
                            lhsT=mt[:],
                            rhs=G[:, cc, :],
                            start=(cc == 0),
                            stop=(cc == ncb - 1),
                        )
                    # agg = s_dst * raw aggregate (dst-side normalization)
                    agg = spool.tile([P, D], f32)
                    nc.scalar.activation(
                        out=agg[:],
                        in_=ps_agg[:],
                        func=mybir.ActivationFunctionType.Copy,
                        scale=sloc_sb[:, b : b + 1],
                    )
                    aggT = tpool.tile([P, 4, P], f32)
                    for j in range(4):
                        ps_t = ptr.tile([P, P], f32)
                        nc.tensor.transpose(
                            out=ps_t[:], in_=agg[:, j * P : (j + 1) * P], identity=ident[:]
                        )
                        nc.vector.tensor_copy(out=aggT[:, j, :], in_=ps_t[:])
                    ps_d = pden.tile([P, D], f32)
                    nc.tensor.matmul(
                        ps_d[:], lhsT=ones1[:], rhs=bsb[:], start=True, stop=False
                    )
                    for j in range(4):
                        nc.tensor.matmul(
                            ps_d[:],
                            lhsT=aggT[:, j, :],
                            rhs=wsb[:, j, :],
                            start=False,
                            stop=(j == 3),
                        )
                    if not is_last:
                        # out = s_dst * relu(dense) == relu(s_dst * dense): the
                        # next layer's src-side normalization, cast to bf16
                        o = spool.tile([P, D], gdt)
                        nc.scalar.activation(
                            out=o[:],
                            in_=ps_d[:],
                            func=mybir.ActivationFunctionType.Relu,
                            scale=sloc_sb[:, b : b + 1],
                        )
                        nc.sync.dma_start(
                            out=out_loc[b * P : b * P + rows, :], in_=o[:rows, :]
                        )
                    else:
                        o = spool.tile([P, D], f32)
                        nc.scalar.activation(
                            out=o[:], in_=ps_d[:], func=mybir.ActivationFunctionType.Relu
                        )
                        nc.sync.dma_start(
                            out=x2_ap[b * P : b * P + rows, :], in_=o[:rows, :]
                        )
                        nc.vector.tensor_tensor(
                            out=mrun[:rows, :],
                            in0=mrun[:rows, :],
                            in1=o[:rows, :],
                            op=mybir.AluOpType.max,
                        )

            for _ in range(rep):
                nc.vector.memset(mrun[:], 0.0)

                # stage 0: xp = s * x (own shard), cast to gather dtype, AllGather
                for b in range(NBLK):
                    rows = P if b < NBLK - 1 else LAST_ROWS
                    xs = spool.tile([P, D], f32)
                    nc.sync.dma_start(
                        out=xs[:rows], in_=x_ap[b * P : b * P + rows, :]
                    )
                    xp = spool.tile([P, D], gdt)
                    nc.scalar.activation(
                        out=xp[:rows],
                        in_=xs[:rows],
                        func=mybir.ActivationFunctionType.Copy,
                        scale=sloc_sb[:rows, b : b + 1],
                    )
                    nc.sync.dma_start(
                        out=xp_loc[b * P : b * P + rows, :], in_=xp[:rows, :]
                    )
                nc.gpsimd.collective_compute(
                    "AllGather",
                    mybir.AluOpType.bypass,
                    replica_groups=RG,
                    ins=[xp_loc[:].opt()],
                    outs=[xp_full[:].opt()],
                )

                layer(xp_full[:], w1sb, b1sb, is_last=False, out_loc=x1p_loc)

                nc.gpsimd.collective_compute(
                    "AllGather",
                    mybir.AluOpType.bypass,
                    replica_groups=RG,
                    ins=[x1p_loc[:].opt()],
                    outs=[x1p_full[:].opt()],
                )

                layer(x1p_full[:], w2sb, b2sb, is_last=True, out_loc=None)

                pm_sb = const.tile([P, 4], f32)
                for j in range(4):
                    ps_t = ptr.tile([P, P], f32)
                    nc.tensor.transpose(
                        out=ps_t[:], in_=mrun[:, j * P : (j + 1) * P], identity=ident[:]
                    )
                    nc.vector.reduce_max(
                        out=pm_sb[:, j : j + 1], in_=ps_t[:], axis=mybir.AxisListType.X
                    )
                nc.sync.dma_start(out=pm_ap[:], in_=pm_sb[:])

    nc.compile()
    return nc


LAST_EXEC_NS = None
LAST_RESULTS = None


def kernel(x, edge_index, W1, b1, W2, b2, _trace=False, _trace_cores=None):
    global LAST_EXEC_NS, LAST_RESULTS
    from concourse.bass_utils import run_bass_kernel_spmd

    x = np.ascontiguousarray(np.asarray(x, dtype=np.float32))
    W1 = np.ascontiguousarray(np.asarray(W1, dtype=np.float32))
    W2 = np.ascontiguousarray(np.asarray(W2, dtype=np.float32))
    b1 = np.ascontiguousarray(np.asarray(b1, dtype=np.float32))
    b2 = np.ascontiguousarray(np.asarray(b2, dtype=np.float32))

    ncb, idx_dev, doff_dev, sloc_dev = _preprocess(edge_index)

    key = (ncb,)
    if key not in _NC_CACHE:
        _NC_CACHE[key] = _build_nc(ncb)
    nc = _NC_CACHE[key]

    in_maps = make_in_maps(x, W1, b1, W2, b2, idx_dev, doff_dev, sloc_dev)

    kwargs = {}
    if _trace:
        kwargs["trace"] = True
        if _trace_cores is not None:
            kwargs["trace_cores"] = _trace_cores
    res = run_bass_kernel_spmd(nc, in_maps, core_ids=list(range(N_CORES)), **kwargs)
    LAST_EXEC_NS = res.exec_time_ns
    LAST_RESULTS = res
    return assemble(res.results)


def make_in_maps(x, W1, b1, W2, b2, idx_dev, doff_dev, sloc_dev):
    in_maps = []
    for c in range(N_CORES):
        in_maps.append(
            {
                "x": np.ascontiguousarray(x[c * NLOC : (c + 1) * NLOC]),
                "W1": W1,
                "W2": W2,
                "b1": b1,
                "b2": b2,
                "gidx": idx_dev[c],
                "doff": doff_dev[c],
                "sloc": sloc_dev[c],
            }
        )
    return in_maps


def assemble(results):
    x2 = np.concatenate([results[c]["x2"] for c in range(N_CORES)], axis=0)
    pm = np.max(np.stack([results[c]["pmax"] for c in range(N_CORES)]), axis=0)
    p = pm.T.reshape(1, D).copy()
    return x2, p
